# revision 47
# baseline (speedup 1.0000x reference)
"""Liquid Neural Network Trainium2 kernel — linearized banded-FIR formulation.

Reference recurrence (tau=1, dt=1):
    h_s = W_hh @ tanh(h_{s-1}) + W_ih @ (W_in @ x_s + b_in) + bias
    y_s = W_out @ tanh(h_s) + b_out

For this problem the hidden state is tiny (|h| < 0.3: input path variance
~0.045, ||W_hh||_2 ~ 0.15), so tanh(h) = h to within ~7.5e-3 relative —
well inside the 2e-2 harness tolerance.  Linearizing BOTH tanh's collapses
the whole recurrent network into a K-tap FIR filter applied directly to x:

    y_s = sum_{k=0..K} c_k . x_{s-k},   c_k^T = W_out A^k G
    (A = (1-1/tau)I + (1/tau)W_hh,  G = (1/tau) W_ih W_in)

||c_k|| decays ~100x per tap, so K=3 suffices (truncation ~1e-4 rel).

Device formulation: a banded (Toeplitz) matmul that keeps all 128 PE rows
and all 128 PSUM partitions productive:
  * x is laid out time-major: 128 consecutive steps per SBUF partition
    column-block, blocks strided by V=128-K steps (K-step overlap).
  * One stationary per input feature i: Band_i[s', m] = c_{m-s'}[i]
    (banded 128x128).  32 accumulating matmul passes (one per i) over all
    block columns produce ALL outputs y[m, (blk, b)] in [128, NBLK*BS]
    PSUM.  No tanh, no scan chain, no per-step copies.
  * PE cost ~14us; x traffic ~8.7MB bf16 -> run is DMA-bound (memory
    regime).  x+bands stream in 2-feature chunks round-robined over FOUR
    DMA queues (SP/Pool/Act/DVE) since one queue tops out ~140GB/s.

8-way data parallel over batch (32 rows per core), same NEFF on all cores.
A post-run self-check compares 3 batch rows against the exact nonlinear
recurrence computed on host; on any mismatch the full exact fallback runs.
"""

import numpy as np

B, I, H = 256, 32, 64
NCORES = 8
BS = B // NCORES                 # batch rows per core
P = 128                          # steps per block (= PE contraction dim)

_nc_cache = {}


def _build_fir(NI, NBLK, bs, chunks):
    """Per-core Bass program; identical NEFF on all cores.

    NI: number of input features (matmul passes); NBLK: time blocks;
    bs: batch rows per core; chunks: tuple of (i0, ni, c0, c1) chunk
    descriptors — features [i0, i0+ni) over output columns [c0, c1).
    """
    import concourse.bacc as bacc
    import concourse.tile as tile
    from concourse import mybir

    nc = bacc.Bacc(
        "TRN2",
        target_bir_lowering=False,
        debug=False,
        enable_asserts=False,
        num_devices=NCORES,
    )
    f32 = mybir.dt.float32
    bf16 = mybir.dt.bfloat16

    YC = NBLK * bs                       # output columns per core
    # chunk DRAM offsets: [band block ni*P | x block ni*(c1-c0)] each
    offs = [0]
    for _, ni, c0, c1 in chunks:
        offs.append(offs[-1] + ni * (P + c1 - c0))
    x_d = nc.dram_tensor("x", [P, offs[-1]], bf16, kind="ExternalInput")
    y_d = nc.dram_tensor("y", [P, YC], bf16, kind="ExternalOutput")
    x_ap = x_d.ap()
    y_ap = y_d.ap()

    # matmul output must stay within one 2KB PSUM bank (512 f32 cols)
    col_splits = [(s, min(s + 512, YC)) for s in range(0, YC, 512)]
    # last chunk writing each bank (its final feature carries stop=True)
    last_for_bank = [
        max(
            ci
            for ci, (_, _, c0, c1) in enumerate(chunks)
            if c0 < s1 and c1 > s0
        )
        for s0, s1 in col_splits
    ]

    with tile.TileContext(nc) as tc:
        with (
            tc.tile_pool(name="consts", bufs=1) as consts,
            tc.tile_pool(name="xpool", bufs=len(chunks)) as xpool,
            tc.tile_pool(name="ypool", bufs=1) as ypool,
            tc.tile_pool(name="ps", bufs=len(col_splits), space="PSUM") as ps,
            tc.tile_pool(name="wps", bufs=1, space="PSUM") as wps,
        ):
            # the two HWDGE queues (SP + Activation) together sustain
            # ~425GB/s; adding the gpsimd SWDGE queue measurably degrades
            # both, so all traffic goes through these two, byte-balanced
            queues = [nc.sync, nc.scalar]
            load = [0.0, 0.0]

            def pick(nbytes, allowed=(0, 1)):
                q = min(allowed, key=lambda j: load[j])
                load[q] += nbytes
                return queues[q]

            # pre-warm the PE clock gate while the first x chunks stream in
            dummy = consts.tile([P, 512], bf16, name="dummy")
            nc.vector.memset(dummy, 0.0)
            wt = wps.tile([P, 512], f32, name="wt")
            for _ in range(8):
                nc.tensor.matmul(
                    wt, dummy[:, :P], dummy,
                    start=True, stop=True, skip_group_check=True,
                )

            # combined band+x chunks: each chunk's stationaries ride at the
            # head of its x DMA so the queue runs few big uniform DMAs
            # (the hardware allows ~4 outstanding per queue).  The final
            # features arrive as two half-column chunks on opposite queues
            # so bank0's y drains while the other banks still accumulate.
            xtiles = []
            prev_q = 1
            for c, (i0, ni, c0, c1) in enumerate(chunks):
                w = c1 - c0
                xt = xpool.tile([P, ni * (P + w)], bf16, name=f"x_sb_{c}", tag="x")
                if c == len(chunks) - 1:
                    # final half-column chunk rides the opposite queue of
                    # its sibling so both arrive in parallel
                    qi = 1 - prev_q
                    load[qi] += ni * (P + w) * 2
                    q = queues[qi]
                else:
                    q = pick(ni * (P + w) * 2)
                    prev_q = queues.index(q)
                q.dma_start(out=xt, in_=x_ap[:, offs[c] : offs[c + 1]])
                xtiles.append(xt)

            ps_tiles = [
                ps.tile([P, 512], f32, name=f"ps_{s0}", tag="ps")[
                    :, : s1 - s0
                ]
                for s0, s1 in col_splits
            ]
            for c, (i0, ni, c0, c1) in enumerate(chunks):
                xt = xtiles[c]
                w = c1 - c0
                for il in range(ni):
                    i = i0 + il
                    for t, (s0, s1) in enumerate(col_splits):
                        if s0 >= c1 or s1 <= c0:
                            continue
                        nc.tensor.matmul(
                            ps_tiles[t],
                            xt[:, il * P : (il + 1) * P],
                            xt[:, ni * P + il * w + s0 - c0 : ni * P + il * w + s1 - c0],
                            start=(i == 0),
                            stop=(c == last_for_bank[t] and il == ni - 1),
                            skip_group_check=True,
                        )

            # y drain: two pieces, one per queue, in parallel.  Piece 0 =
            # bank 0; piece 1 = the remaining banks glued into one tile.
            p1lo = col_splits[0][1]
            y_sb0 = ypool.tile([P, p1lo], bf16, name="y_sb0", tag="y0")
            nc.vector.tensor_copy(out=y_sb0, in_=ps_tiles[0])
            queues[0].dma_start(out=y_ap[:, :p1lo], in_=y_sb0)
            if YC > p1lo:
                y_sb1 = ypool.tile([P, YC - p1lo], bf16, name="y_sb1", tag="y1")
                for t, (s0, s1) in enumerate(col_splits[1:], start=1):
                    nc.vector.tensor_copy(
                        out=y_sb1[:, s0 - p1lo : s1 - p1lo], in_=ps_tiles[t]
                    )
                queues[1].dma_start(out=y_ap[:, p1lo:], in_=y_sb1)

    nc.compile()
    return nc


def _numpy_fallback(x, W_in, b_in, W_hh, W_ih, bias, tau, W_out, b_out):
    x = np.asarray(x, np.float32)
    nbatch, n_steps, _ = x.shape
    hid = W_hh.shape[0]
    u = x @ np.asarray(W_in, np.float32).T + np.asarray(b_in, np.float32)
    ie = u @ np.asarray(W_ih, np.float32).T
    tau = np.asarray(tau, np.float32)
    bias = np.asarray(bias, np.float32)
    W_hhT = np.asarray(W_hh, np.float32).T
    W_outT = np.asarray(W_out, np.float32).T
    h = np.zeros((nbatch, hid), np.float32)
    out = np.empty((nbatch, n_steps, W_outT.shape[1]), np.float32)
    for s in range(n_steps):
        dhdt = (-h + np.tanh(h) @ W_hhT + ie[:, s] + bias) / tau
        h = h + dhdt
        out[:, s] = np.tanh(h) @ W_outT
    return out + np.asarray(b_out, np.float32)


def kernel(x, W_in, b_in, W_hh, W_ih, bias, tau, W_out, b_out):
    import ml_dtypes

    x = np.asarray(x, np.float32)
    nbatch, S, nin = x.shape
    nh = W_hh.shape[0]
    nout = W_out.shape[0]

    tau64 = np.asarray(tau, np.float64)
    W_in64 = np.asarray(W_in, np.float64)
    W_ih64 = np.asarray(W_ih, np.float64)
    W_hh64 = np.asarray(W_hh, np.float64)
    b_in64 = np.asarray(b_in, np.float64)
    bias64 = np.asarray(bias, np.float64)
    W_out64 = np.asarray(W_out, np.float64)
    b_out64 = np.asarray(b_out, np.float64)

    if np.any(tau64 <= 0) or nout != 1 or nbatch % NCORES != 0:
        return _numpy_fallback(x, W_in, b_in, W_hh, W_ih, bias, tau, W_out, b_out)

    inv = 1.0 / tau64
    A = np.diag(1.0 - inv) + inv[:, None] * W_hh64        # h_s = A h + G x + beta
    G = inv[:, None] * (W_ih64 @ W_in64)
    beta = inv * (W_ih64 @ b_in64 + bias64)

    sigma = float(np.linalg.norm(A, 2))
    bs = nbatch // NCORES
    # static gates: decay fast enough for <=8 taps, state small enough that
    # tanh ~ id holds; anything else -> exact fallback
    hscale = float(np.linalg.norm(G)) / max(1e-9, 1.0 - sigma)
    if sigma > 0.55 or hscale > 0.6 or nin > 128 or bs > 64 or S < 256:
        return _numpy_fallback(x, W_in, b_in, W_hh, W_ih, bias, tau, W_out, b_out)

    # FIR taps c_k = W_out A^k G  [K+1, nin]
    taps = []
    M = np.eye(nh)
    c0n = max(1e-30, float(np.linalg.norm(W_out64 @ G)))
    K = 0
    for k in range(9):
        taps.append((W_out64 @ M @ G)[0])
        M = A @ M
        K = k
        if float(np.linalg.norm(W_out64 @ M @ G)) < 2e-4 * c0n:
            break
    else:
        return _numpy_fallback(x, W_in, b_in, W_hh, W_ih, bias, tau, W_out, b_out)
    C = np.array(taps)                                    # [K+1, nin]

    # constant offset from bias path: y_off[s] = W_out . sum_{j<s+1} A^j beta
    yoff = np.zeros(S)
    if np.any(beta != 0):
        acc = np.zeros(nh)
        v = beta.copy()
        pos = np.empty(min(S, 200))
        for s in range(len(pos)):
            acc = acc + v
            pos[s] = float((W_out64 @ acc).reshape(-1)[0])
            v = A @ v
        yoff[: len(pos)] = pos
        yoff[len(pos):] = pos[-1]
    yoff += float(b_out64.reshape(-1)[0])

    V = P - K                                             # valid outputs/block
    PLEN = S + K                                          # zero-padded steps
    NBLK = max(1, -(-(PLEN - P) // V) + 1) if PLEN > P else 1
    while (NBLK - 1) * V + P < PLEN:
        NBLK += 1
    NI = nin

    # Chunk plan: uniform 4-feature chunks (~9.5KB DMA descriptors, 4 DMAs
    # per queue = the hardware outstanding window).  Measured alternatives
    # (8-feature, fine tails, column-split tails) were all equal or slower:
    # lone small transfers drain latency-bound at ~1.5-2us apiece.
    YCt = NBLK * bs
    if NI >= 12:
        sizes = (NI - 4 * (NI // 4 - 1),) + (4,) * (NI // 4 - 1)
    else:
        sizes = (NI,)
    chunks = []
    i0 = 0
    for ni in sizes:
        chunks.append((i0, ni, 0, YCt))
        i0 += ni
    chunks = tuple(chunks)

    key = (NI, NBLK, bs, chunks)
    if key not in _nc_cache:
        _nc_cache[key] = _build_fir(NI, NBLK, bs, chunks)
    nc = _nc_cache[key]

    # banded stationaries: band[s', i*P + m] = C[m-s', i] for 0<=m-s'<=K
    band = np.zeros((P, NI, P), np.float32)
    for k in range(K + 1):
        sp = np.arange(P - k)
        band[sp, :, sp + k] = C[k][None, :].repeat(P - k, axis=0)
    band = band.reshape(P, NI * P).astype(ml_dtypes.bfloat16)

    # x -> per-core time-major blocked layout [P, (i, blk, b)], interleaved
    # per chunk as [band block | x block] so each chunk is a single DMA
    pad_tail = (NBLK - 1) * V + P - PLEN
    YC = NBLK * bs
    tot = sum(ni * (P + c1 - c0) for _, ni, c0, c1 in chunks)
    in_maps = []
    xp = np.zeros((nbatch, PLEN + pad_tail, nin), np.float32)
    xp[:, K : K + S] = x
    win = np.lib.stride_tricks.sliding_window_view(xp, P, axis=1)[:, ::V]
    # win: [nbatch, NBLK, nin, P]
    for c in range(NCORES):
        wc = win[c * bs : (c + 1) * bs]                   # [bs, NBLK, nin, P]
        xdev = np.ascontiguousarray(
            wc.transpose(3, 2, 1, 0).reshape(P, NI * NBLK * bs)
        ).astype(ml_dtypes.bfloat16)
        xb = np.empty((P, tot), ml_dtypes.bfloat16)
        off = 0
        for i0, ni, c0, c1 in chunks:
            w = c1 - c0
            xb[:, off : off + ni * P] = band[:, i0 * P : (i0 + ni) * P]
            for il in range(ni):
                xb[:, off + ni * P + il * w : off + ni * P + (il + 1) * w] = (
                    xdev[:, (i0 + il) * YC + c0 : (i0 + il) * YC + c1]
                )
            off += ni * (P + w)
        in_maps.append({"x": np.ascontiguousarray(xb)})

    from concourse.bass_utils import run_bass_kernel_spmd

    res = run_bass_kernel_spmd(nc, in_maps, core_ids=list(range(NCORES)))
    kernel.last_results = res

    y = np.empty((nbatch, S, 1), np.float32)
    for c in range(NCORES):
        yr = np.asarray(res.results[c]["y"], np.float32).reshape(P, NBLK, bs)
        v = yr[K:].transpose(1, 0, 2).reshape(NBLK * V, bs)[:S]   # [S, bs]
        y[c * bs : (c + 1) * bs, :, 0] = v.T
    y += yoff.astype(np.float32)[None, :, None]

    # self-check 3 rows against the exact nonlinear recurrence
    rows = sorted({0, nbatch // 2, nbatch - 1})
    y_ex = _numpy_fallback(
        x[rows], W_in, b_in, W_hh, W_ih, bias, tau, W_out, b_out
    )
    scale = max(1e-30, float(np.abs(y_ex).max()))
    rel = float(np.abs(y[rows] - y_ex).max()) / scale
    if not np.isfinite(rel) or rel > 1.4e-2:
        return _numpy_fallback(x, W_in, b_in, W_hh, W_ih, bias, tau, W_out, b_out)
    return y


kernel.last_results = None
